# revision 20
# baseline (speedup 1.0000x reference)
"""Trainium2 Bass kernel for ProductionTPA (sparse prototype attention).

Strategy (data-parallel over B, one batch element per NeuronCore):
  - LayerNorm stats via bn_stats in natural [t,d] layout; normalize with a
    single fused tensor_scalar (x*rstd + (-mu*rstd)).  ln_g/ln_b are folded
    into the weight matrices on the host (exact, O(D^2) work).
  - xn is transposed 128x128-blockwise on the PE so all D-contractions run
    on the tensor engine at full rate (float32r: fp32 storage, 1 cyc/row).
  - K is computed in transposed layout per head only to obtain per-token
    L2 norms: ACT squares K (PSUM->SBUF bf16), a ones-matmul reduces over
    the partition axis, giving nrm2 directly in [h, t] layout.
  - Scores are computed WITHOUT using K: the tiny matrix Mq = (Wk_h^T
    Qh_h)/TEMP is folded on the host, so scores_T = Mq^T @ xn_T.
  - Softmax over t needs no max subtraction: |logit| <= 1/TEMP by
    Cauchy-Schwarz (Q and K are L2-normalized), well within fp32 range.
    The denominator comes free via the activation accum_out.
  - The V projection is deferred: Y = E @ xn is accumulated in PSUM over
    the whole sequence, then projected once by Wv at the end (saves the
    full [T,D]x[D,D] V matmul).
  - top-k via rank counting (compare matrix -> rank -> mask), z = mask^T @
    proto_tokens / k, then a tiny fp32 MLP.
"""

import sys

sys.path.insert(0, "/opt/trn_rl_repo")

from contextlib import ExitStack

import numpy as np

import concourse.bass as bass
import concourse.tile as tile
from concourse import bacc
from concourse import mybir
from concourse.bass_utils import run_bass_kernel_spmd

F32 = mybir.dt.float32
F32R = mybir.dt.float32r
BF16 = mybir.dt.bfloat16
AF = mybir.ActivationFunctionType
OP = mybir.AluOpType

B, T, D, H, P, HD = 8, 8192, 512, 4, 24, 128
HP = H * P  # 96
TEMP = 0.07
TOPK = 12
LN_EPS = 1e-5
SUB = 512  # tokens per subchunk
NBLK = SUB // 128  # 4
NDC = D // 128  # 4


def r(ap):
    return ap.bitcast(F32R)


def build(nc, t_total=T):
    nsub = t_total // SUB

    def din(name, shape, dt=F32):
        return nc.dram_tensor(name, shape, dt, kind="ExternalInput").ap()

    xb = din("xb", [t_total, D])
    wkT = din("wkT", [D, D], F32R)
    mq = din("mq", [D, HP], F32R)
    sbias = din("sbias", [HP, 1])
    biask = din("biask", [128, H])
    wvT = din("wvT", [D, D], F32R)
    biasv = din("biasv", [1, D], F32R)
    w1T = din("w1T", [D, D])
    b1T = din("b1T", [128, NDC])
    w2T = din("w2T", [D, D])
    b2row = din("b2row", [1, D])
    ident = din("ident", [128, 128], F32R)
    selk = din("selk", [128, H, HP], BF16)
    ones24 = din("ones24", [1, P], F32R)
    ones1 = din("ones1", [1, 1], F32R)
    onesPP = din("onesPP", [P, P])
    y = nc.dram_tensor("y", [1, D], F32, kind="ExternalOutput").ap()

    xr = xb.rearrange("(n p) d -> p n d", p=128)  # [128, t/128, D]

    with tile.TileContext(nc) as tc, ExitStack() as ctx:
        wp = ctx.enter_context(tc.tile_pool(name="wp", bufs=1))
        yp = ctx.enter_context(tc.tile_pool(name="yp", bufs=1, space="PSUM"))
        lp = ctx.enter_context(tc.tile_pool(name="lp", bufs=1))

        # resident weights
        wkT_sb = wp.tile([128, NDC, D], F32R)
        nc.sync.dma_start(wkT_sb, wkT.rearrange("(c p) j -> p c j", p=128))
        mq_sb = wp.tile([128, NDC, HP], F32R)
        nc.sync.dma_start(mq_sb, mq.rearrange("(c p) j -> p c j", p=128))
        wvT_sb = wp.tile([128, NDC, D], F32R)
        nc.sync.dma_start(wvT_sb, wvT.rearrange("(c p) j -> p c j", p=128))
        w1T_sb = wp.tile([128, NDC, D], F32)
        nc.sync.dma_start(w1T_sb, w1T.rearrange("(c p) j -> p c j", p=128))
        w2T_sb = wp.tile([128, NDC, D], F32)
        nc.sync.dma_start(w2T_sb, w2T.rearrange("(c p) j -> p c j", p=128))
        ident_sb = wp.tile([128, 128], F32R)
        nc.sync.dma_start(ident_sb, ident)
        selk_sb = wp.tile([128, H, HP], BF16)
        nc.sync.dma_start(selk_sb, selk)
        sbias_sb = wp.tile([HP, 1], F32)
        nc.sync.dma_start(sbias_sb, sbias)
        biask_sb = wp.tile([128, H], F32)
        nc.sync.dma_start(biask_sb, biask)
        biasv_sb = wp.tile([1, D], F32R)
        nc.sync.dma_start(biasv_sb, biasv)
        b1T_sb = wp.tile([128, NDC], F32)
        nc.sync.dma_start(b1T_sb, b1T)
        b2row_sb = wp.tile([1, D], F32)
        nc.sync.dma_start(b2row_sb, b2row)
        ones24_sb = wp.tile([1, P], F32R)
        nc.sync.dma_start(ones24_sb, ones24)
        ones1_sb = wp.tile([1, 1], F32R)
        nc.sync.dma_start(ones1_sb, ones1)
        onesPP_sb = wp.tile([P, P], F32)
        nc.sync.dma_start(onesPP_sb, onesPP)

        eps_sb = wp.tile([128, 1], F32)
        nc.vector.memset(eps_sb, LN_EPS)

        y_ps = yp.tile([HP, D], F32)  # attention-weighted xn sums, PSUM-resident
        l_parts = lp.tile([HP, nsub], F32)  # per-subchunk softmax denominators

        with ExitStack() as sc_ctx:
            xp = sc_ctx.enter_context(tc.tile_pool(name="xp", bufs=3))
            sp = sc_ctx.enter_context(tc.tile_pool(name="sp", bufs=2))
            xnp = sc_ctx.enter_context(tc.tile_pool(name="xnp", bufs=2))
            xtp = sc_ctx.enter_context(tc.tile_pool(name="xtp", bufs=2))
            ksp = sc_ctx.enter_context(tc.tile_pool(name="ksp", bufs=2))
            rnp = sc_ctx.enter_context(tc.tile_pool(name="rnp", bufs=2))
            ep = sc_ctx.enter_context(tc.tile_pool(name="ep", bufs=2))
            etp = sc_ctx.enter_context(tc.tile_pool(name="etp", bufs=2))
            ktps = sc_ctx.enter_context(tc.tile_pool(name="ktps", bufs=2, space="PSUM"))
            scps = sc_ctx.enter_context(tc.tile_pool(name="scps", bufs=1, space="PSUM"))
            bnps = sc_ctx.enter_context(tc.tile_pool(name="bnps", bufs=2, space="PSUM"))
            trps = sc_ctx.enter_context(tc.tile_pool(name="trps", bufs=2, space="PSUM"))

            for s in range(nsub):
                x_sub = xp.tile([128, NBLK, D], F32)
                nc.sync.dma_start(x_sub, xr[:, s * NBLK : (s + 1) * NBLK, :])

                # --- LayerNorm stats ---
                stats = sp.tile([128, NBLK, 6], F32, tag="stats")
                for bb in range(NBLK):
                    nc.vector.bn_stats(stats[:, bb, :], x_sub[:, bb, :])
                mv = sp.tile([128, NBLK, 2], F32, tag="mv")
                for bb in range(NBLK):
                    nc.vector.bn_aggr(mv[:, bb, :], stats[:, bb, :])
                srt = sp.tile([128, NBLK], F32, tag="srt")
                nc.scalar.activation(srt, mv[:, :, 1], AF.Sqrt, bias=eps_sb)
                rstd = sp.tile([128, NBLK], F32, tag="rstd")
                nc.vector.reciprocal(rstd, srt)
                nmr = sp.tile([128, NBLK], F32, tag="nmr")
                nc.vector.scalar_tensor_tensor(
                    nmr, mv[:, :, 0], -1.0, rstd, op0=OP.mult, op1=OP.mult
                )

                # --- normalize: xn0 = x*rstd + (-mu*rstd) ---
                xn0 = xnp.tile([128, NBLK, D], F32R)
                for bb in range(NBLK):
                    nc.vector.tensor_scalar(
                        xn0[:, bb, :],
                        x_sub[:, bb, :],
                        rstd[:, bb : bb + 1],
                        nmr[:, bb : bb + 1],
                        op0=OP.mult,
                        op1=OP.add,
                    )

                # --- transpose xn0 -> xn_T [d-part, t-free] ---
                xn_T = xtp.tile([128, NDC, SUB], F32R)
                for bb in range(NBLK):
                    tr = trps.tile([128, NDC, 128], F32R, tag="tr")
                    for dc in range(NDC):
                        nc.tensor.matmul(
                            r(tr[:, dc, :]),
                            r(xn0[:, bb, dc * 128 : (dc + 1) * 128]),
                            r(ident_sb),
                            is_transpose=True,
                            start=(dc == 0),
                            stop=(dc == NDC - 1),
                        )
                    dst = xn_T[:, :, bb * 128 : (bb + 1) * 128]
                    if bb % 2 == 0:
                        nc.vector.tensor_copy(dst, tr)
                    else:
                        nc.scalar.copy(dst, tr)

                # --- K_T per head (PSUM) -> Ksq (bf16, SBUF) ---
                ksq = ksp.tile([128, H, SUB], BF16)
                for h in range(H):
                    kt = ktps.tile([128, SUB], F32, tag="kt")
                    for dc in range(NDC):
                        nc.tensor.matmul(
                            kt,
                            r(wkT_sb[:, dc, h * 128 : (h + 1) * 128]),
                            r(xn_T[:, dc, :]),
                            start=(dc == 0),
                            stop=(dc == NDC - 1),
                        )
                    nc.scalar.activation(
                        ksq[:, h, :], kt, AF.Square, bias=biask_sb[:, h : h + 1]
                    )

                # --- nrm2 reduced over partitions AND broadcast to [HP, SUB]
                # in one accumulating matmul with block-column ones masks ---
                bc_n2 = bnps.tile([HP, SUB], F32)
                for h in range(H):
                    nc.tensor.matmul(
                        bc_n2,
                        selk_sb[:, h, :],
                        ksq[:, h, :],
                        start=(h == 0),
                        stop=(h == H - 1),
                    )
                rn_r = rnp.tile([HP, SUB], F32, tag="rnr")
                nc.vector.reciprocal(rn_r, bc_n2)
                rn = rnp.tile([HP, SUB], F32, tag="rn")
                nc.scalar.activation(rn, rn_r, AF.Sqrt)

                # --- scores_T = Mq^T @ xn_T (PSUM) ---
                scp = scps.tile([HP, SUB], F32)
                for dc in range(NDC):
                    nc.tensor.matmul(
                        scp,
                        r(mq_sb[:, dc, :]),
                        r(xn_T[:, dc, :]),
                        start=(dc == 0),
                        stop=(dc == NDC - 1),
                    )

                # --- logits = (scores + sbias) * rnorm ; E = exp ---
                sE = ep.tile([HP, SUB], F32, tag="se")
                nc.vector.scalar_tensor_tensor(
                    sE, scp, sbias_sb, rn, op0=OP.add, op1=OP.mult
                )
                E = ep.tile([HP, SUB], F32R, tag="e")
                nc.scalar.activation(
                    E, sE, AF.Exp, accum_out=l_parts[:, s : s + 1]
                )

                # --- E_T and Y accumulation ---
                et_ps = trps.tile([128, NBLK, HP], F32R, tag="tr")
                for bb in range(NBLK):
                    nc.tensor.matmul(
                        r(et_ps[:, bb, :]),
                        r(E[:, bb * 128 : (bb + 1) * 128]),
                        r(ident_sb[:HP, :HP]),
                        is_transpose=True,
                        start=(bb == 0),
                        stop=(bb == NBLK - 1),
                    )
                ET = etp.tile([128, NBLK, HP], F32R)
                nc.vector.tensor_copy(ET, et_ps)
                for bb in range(NBLK):
                    nc.tensor.matmul(
                        y_ps,
                        r(ET[:, bb, :]),
                        r(xn0[:, bb, :]),
                        start=(s == 0 and bb == 0),
                        stop=(s == nsub - 1 and bb == NBLK - 1),
                    )

        # ---------------- final stage ----------------
        with ExitStack() as f_ctx:
            fp = f_ctx.enter_context(tc.tile_pool(name="fp", bufs=1))
            fps = f_ctx.enter_context(tc.tile_pool(name="fps", bufs=1, space="PSUM"))

            lsum = fp.tile([HP, 1], F32, tag="lsum")
            nc.vector.tensor_reduce(lsum, l_parts, axis=mybir.AxisListType.X, op=OP.add)
            linv = fp.tile([HP, 1], F32, tag="linv")
            nc.vector.reciprocal(linv, lsum)
            yn = fp.tile([HP, D], F32R, tag="yn")
            nc.vector.tensor_scalar(yn, y_ps, linv, None, op0=OP.mult)

            ynt_ps = fps.tile([128, NDC, HP], F32R, tag="f1")
            for dc in range(NDC):
                nc.tensor.matmul(
                    r(ynt_ps[:, dc, :]),
                    r(yn[:, dc * 128 : (dc + 1) * 128]),
                    r(ident_sb[:HP, :HP]),
                    is_transpose=True,
                    start=(dc == 0),
                    stop=(dc == NDC - 1),
                )
            ynt = fp.tile([128, NDC, HP], F32R, tag="ynt")
            nc.vector.tensor_copy(ynt, ynt_ps)

            # proto_tokens [P, D] = Yn @ WvT (per head) + bias_v
            proto_ps = fps.tile([P, D], F32, tag="f2")
            for h in range(H):
                for dc in range(NDC):
                    nc.tensor.matmul(
                        proto_ps[:, h * 128 : (h + 1) * 128],
                        r(ynt[:, dc, h * P : (h + 1) * P]),
                        r(wvT_sb[:, dc, h * 128 : (h + 1) * 128]),
                        start=(h == 0 and dc == 0),
                        stop=False,
                    )
            nc.tensor.matmul(proto_ps, r(ones24_sb), r(biasv_sb), start=False, stop=True)
            proto_sb = fp.tile([P, D], F32, tag="proto")
            nc.vector.tensor_copy(proto_sb, proto_ps)

            # proto_scores + rank-count top-k mask
            junk = fp.tile([P, D], F32, tag="junk")
            psc = fp.tile([P, 1], F32, tag="psc")
            nc.vector.scalar_tensor_tensor(
                junk, proto_sb, 1.0, proto_sb, op0=OP.mult, op1=OP.mult, accum_out=psc
            )
            pscT_ps = fps.tile([1, P], F32, tag="f3")
            nc.tensor.transpose(pscT_ps, psc, ident_sb[:P, :P].bitcast(F32))
            pscT = fp.tile([1, P], F32, tag="psct")
            nc.vector.tensor_copy(pscT, pscT_ps)
            g_ps = fps.tile([P, P], F32, tag="f4")
            nc.tensor.matmul(g_ps, ones24_sb.bitcast(F32), pscT, start=True, stop=True)
            junk2 = fp.tile([P, P], F32, tag="junk2")
            rank = fp.tile([P, 1], F32, tag="rank")
            nc.vector.scalar_tensor_tensor(
                junk2, g_ps, psc, onesPP_sb, op0=OP.is_gt, op1=OP.mult, accum_out=rank
            )
            mask = fp.tile([P, 1], F32, tag="mask")
            nc.vector.tensor_scalar(
                mask, rank, float(TOPK) - 0.5, None, op0=OP.is_lt
            )

            # z_T [128, NDC] = (proto^T @ mask) / TOPK
            z_ps = fps.tile([128, NDC], F32, tag="f5")
            for dc in range(NDC):
                nc.tensor.matmul(
                    z_ps[:, dc : dc + 1],
                    proto_sb[:, dc * 128 : (dc + 1) * 128],
                    mask,
                    start=(dc == 0),
                    stop=(dc == NDC - 1),
                )
            zT = fp.tile([128, NDC], F32, tag="zt")
            nc.scalar.activation(zT, z_ps, AF.Copy, scale=1.0 / TOPK)

            # h1_T [128, NDC] = W1 @ z  (columnwise)
            h1_ps = fps.tile([128, NDC], F32, tag="f6")
            for jc in range(NDC):
                for dc in range(NDC):
                    nc.tensor.matmul(
                        h1_ps[:, jc : jc + 1],
                        w1T_sb[:, dc, jc * 128 : (jc + 1) * 128],
                        zT[:, dc : dc + 1],
                        start=(jc == 0 and dc == 0),
                        stop=(jc == NDC - 1 and dc == NDC - 1),
                    )
            h1b = fp.tile([128, NDC], F32, tag="h1b")
            nc.vector.tensor_add(h1b, h1_ps, b1T_sb)
            h1g = fp.tile([128, NDC], F32, tag="h1g")
            nc.scalar.activation(h1g, h1b, AF.Sigmoid)
            h1s = fp.tile([128, NDC], F32, tag="h1s")
            nc.vector.tensor_mul(h1s, h1b, h1g)

            # out_row [1, D] = h1 @ W2^T + b2
            out_ps = fps.tile([1, D], F32, tag="f7")
            for dc in range(NDC):
                nc.tensor.matmul(
                    out_ps,
                    h1s[:, dc : dc + 1],
                    w2T_sb[:, dc, :],
                    start=(dc == 0),
                    stop=False,
                )
            nc.tensor.matmul(out_ps, ones1_sb.bitcast(F32), b2row_sb, start=False, stop=True)
            out_sb = fp.tile([1, D], F32, tag="out")
            nc.scalar.activation(out_sb, out_ps, AF.Copy)
            nc.sync.dma_start(y, out_sb)

    return nc


def make_host_weights(proto, ln_g, ln_b, Wq, Wk, Wv, W1, b1, W2, b2):
    f8 = np.float64
    g = ln_g.astype(f8)
    b = ln_b.astype(f8)
    Wk8, Wv8, Wq8 = Wk.astype(f8), Wv.astype(f8), Wq.astype(f8)
    Wk_eff = Wk8 * g[None, :]
    bias_k = Wk8 @ b
    Wv_eff = Wv8 * g[None, :]
    bias_v = Wv8 @ b
    Qp = proto.astype(f8) @ Wq8.T
    Qh = Qp.reshape(P, H, HD).transpose(1, 0, 2)
    nrm = np.maximum(np.linalg.norm(Qh, axis=-1, keepdims=True), 1e-12)
    Qh = Qh / nrm
    Mq = np.zeros((D, HP), f8)
    sbias = np.zeros((HP, 1), f8)
    for h in range(H):
        Mq[:, h * P : (h + 1) * P] = (Qh[h] @ Wk_eff[h * HD : (h + 1) * HD]).T / TEMP
        sbias[h * P : (h + 1) * P, 0] = Qh[h] @ bias_k[h * HD : (h + 1) * HD] / TEMP
    import ml_dtypes

    f = np.float32
    return {
        "wkT": np.ascontiguousarray(Wk_eff.T, f),
        "mq": np.ascontiguousarray(Mq, f),
        "sbias": sbias.astype(f),
        "biask": np.ascontiguousarray(bias_k.reshape(H, 128).T, f),
        "wvT": np.ascontiguousarray(Wv_eff.T, f),
        "biasv": np.ascontiguousarray(bias_v[None, :], f),
        "w1T": np.ascontiguousarray(W1.astype(f8).T, f),
        "b1T": np.ascontiguousarray(b1.astype(f8).reshape(NDC, 128).T, f),
        "w2T": np.ascontiguousarray(W2.astype(f8).T, f),
        "b2row": np.ascontiguousarray(b2.astype(f8)[None, :], f),
        "ident": np.eye(128, dtype=f),
        "selk": np.ascontiguousarray(
            np.broadcast_to(
                np.repeat(np.eye(H), P, axis=1)[None, :, :], (128, H, HP)
            ).astype(ml_dtypes.bfloat16)
        ),
        "ones24": np.ones((1, P), f),
        "ones1": np.ones((1, 1), f),
        "onesPP": np.ones((P, P), f),
    }


def _install_ntff_hook():
    """Recreate the missing antenv.axon_hooks registry and register the
    ctypes NTFF profile hook so run_bass_kernel_spmd(trace=True) works."""
    import types

    try:
        import antenv
    except ImportError:
        return False
    if "antenv.axon_hooks" not in sys.modules:
        m = types.ModuleType("antenv.axon_hooks")
        m._hook = None
        m.set_axon_ntff_profile_hook = lambda h: setattr(m, "_hook", h)
        m.get_axon_ntff_profile_hook = lambda: m._hook
        sys.modules["antenv.axon_hooks"] = m
        antenv.axon_hooks = m
    m = sys.modules["antenv.axon_hooks"]
    if m.get_axon_ntff_profile_hook() is None:
        try:
            from trn_agent_boot.trn_boot import _ntff_profile_via_ctypes

            hook = _ntff_profile_via_ctypes("/opt/axon/libaxon_pjrt.so")
            if hook is not None:
                m.set_axon_ntff_profile_hook(hook)
        except Exception as e:  # noqa: BLE001
            print("ntff hook install failed:", e)
            return False
    return m.get_axon_ntff_profile_hook() is not None


def kernel(
    x, proto, ln_g, ln_b, Wq, Wk, Wv, W1, b1, W2, b2, _trace=False, _t_total=T
):
    x = np.ascontiguousarray(np.asarray(x, np.float32))
    shared = make_host_weights(
        *(np.asarray(a, np.float32) for a in (proto, ln_g, ln_b, Wq, Wk, Wv, W1, b1, W2, b2))
    )
    nc = bacc.Bacc("TRN2", target_bir_lowering=False, debug=False, enable_asserts=False)
    build(nc, _t_total)
    nc.compile()
    in_maps = [dict(shared, xb=np.ascontiguousarray(x[bi])) for bi in range(B)]
    if _trace:
        _trace = _install_ntff_hook()
    res = run_bass_kernel_spmd(nc, in_maps, core_ids=list(range(B)), trace=_trace)
    out = np.stack([res.results[c]["y"][0] for c in range(B)])
    if _trace:
        kernel.last_results = res
    return out


# revision 21
# speedup vs baseline: 1.1212x; 1.1212x over previous
"""Trainium2 Bass kernel for ProductionTPA (sparse prototype attention).

Strategy (data-parallel over B, one batch element per NeuronCore):
  - LayerNorm stats via bn_stats in natural [t,d] layout; normalize with a
    single fused tensor_scalar (x*rstd + (-mu*rstd)).  ln_g/ln_b are folded
    into the weight matrices on the host (exact, O(D^2) work).
  - xn is transposed 128x128-blockwise on the PE so all D-contractions run
    on the tensor engine at full rate (float32r: fp32 storage, 1 cyc/row).
  - K is computed in transposed layout per head only to obtain per-token
    L2 norms: ACT squares K (PSUM->SBUF bf16), a ones-matmul reduces over
    the partition axis, giving nrm2 directly in [h, t] layout.
  - Scores are computed WITHOUT using K: the tiny matrix Mq = (Wk_h^T
    Qh_h)/TEMP is folded on the host, so scores_T = Mq^T @ xn_T.
  - Softmax over t needs no max subtraction: |logit| <= 1/TEMP by
    Cauchy-Schwarz (Q and K are L2-normalized), well within fp32 range.
    The denominator comes free via the activation accum_out.
  - The V projection is deferred: Y = E @ xn is accumulated in PSUM over
    the whole sequence, then projected once by Wv at the end (saves the
    full [T,D]x[D,D] V matmul).
  - top-k via rank counting (compare matrix -> rank -> mask), z = mask^T @
    proto_tokens / k, then a tiny fp32 MLP.
"""

import sys

sys.path.insert(0, "/opt/trn_rl_repo")

from contextlib import ExitStack

import numpy as np

import concourse.bass as bass
import concourse.tile as tile
from concourse import bacc
from concourse import mybir
from concourse.bass_utils import run_bass_kernel_spmd

F32 = mybir.dt.float32
F32R = mybir.dt.float32r
BF16 = mybir.dt.bfloat16
AF = mybir.ActivationFunctionType
OP = mybir.AluOpType

B, T, D, H, P, HD = 8, 8192, 512, 4, 24, 128
HP = H * P  # 96
TEMP = 0.07
TOPK = 12
LN_EPS = 1e-5
SUB = 512  # tokens per subchunk
NBLK = SUB // 128  # 4
NDC = D // 128  # 4


def r(ap):
    return ap.bitcast(F32R)


def build(nc, t_total=T):
    nsub = t_total // SUB

    def din(name, shape, dt=F32):
        return nc.dram_tensor(name, shape, dt, kind="ExternalInput").ap()

    xb = din("xb", [t_total, D])
    wkT = din("wkT", [D, D], F32R)
    mq = din("mq", [D, HP], F32R)
    sbias = din("sbias", [HP, 1])
    biask = din("biask", [128, H])
    wvT = din("wvT", [D, D], F32R)
    biasv = din("biasv", [1, D], F32R)
    w1T = din("w1T", [D, D])
    b1T = din("b1T", [128, NDC])
    w2T = din("w2T", [D, D])
    b2row = din("b2row", [1, D])
    ident = din("ident", [128, 128], F32R)
    selk = din("selk", [128, H, HP], BF16)
    ones24 = din("ones24", [1, P], F32R)
    ones1 = din("ones1", [1, 1], F32R)
    onesPP = din("onesPP", [P, P])
    y = nc.dram_tensor("y", [1, D], F32, kind="ExternalOutput").ap()

    xr = xb.rearrange("(n p) d -> p n d", p=128)  # [128, t/128, D]

    with tile.TileContext(nc) as tc, ExitStack() as ctx:
        wp = ctx.enter_context(tc.tile_pool(name="wp", bufs=1))
        yp = ctx.enter_context(tc.tile_pool(name="yp", bufs=1, space="PSUM"))
        lp = ctx.enter_context(tc.tile_pool(name="lp", bufs=1))

        # resident weights
        wkT_sb = wp.tile([128, NDC, D], F32R)
        nc.sync.dma_start(wkT_sb, wkT.rearrange("(c p) j -> p c j", p=128))
        mq_sb = wp.tile([128, NDC, HP], F32R)
        nc.sync.dma_start(mq_sb, mq.rearrange("(c p) j -> p c j", p=128))
        wvT_sb = wp.tile([128, NDC, D], F32R)
        nc.sync.dma_start(wvT_sb, wvT.rearrange("(c p) j -> p c j", p=128))
        w1T_sb = wp.tile([128, NDC, D], F32)
        nc.sync.dma_start(w1T_sb, w1T.rearrange("(c p) j -> p c j", p=128))
        w2T_sb = wp.tile([128, NDC, D], F32)
        nc.sync.dma_start(w2T_sb, w2T.rearrange("(c p) j -> p c j", p=128))
        ident_sb = wp.tile([128, 128], F32R)
        nc.sync.dma_start(ident_sb, ident)
        selk_sb = wp.tile([128, H, HP], BF16)
        nc.sync.dma_start(selk_sb, selk)
        sbias_sb = wp.tile([HP, 1], F32)
        nc.sync.dma_start(sbias_sb, sbias)
        biask_sb = wp.tile([128, H], F32)
        nc.sync.dma_start(biask_sb, biask)
        biasv_sb = wp.tile([1, D], F32R)
        nc.sync.dma_start(biasv_sb, biasv)
        b1T_sb = wp.tile([128, NDC], F32)
        nc.sync.dma_start(b1T_sb, b1T)
        b2row_sb = wp.tile([1, D], F32)
        nc.sync.dma_start(b2row_sb, b2row)
        ones24_sb = wp.tile([1, P], F32R)
        nc.sync.dma_start(ones24_sb, ones24)
        ones1_sb = wp.tile([1, 1], F32R)
        nc.sync.dma_start(ones1_sb, ones1)
        onesPP_sb = wp.tile([P, P], F32)
        nc.sync.dma_start(onesPP_sb, onesPP)

        eps_sb = wp.tile([128, 1], F32)
        nc.vector.memset(eps_sb, LN_EPS)

        y_ps = yp.tile([HP, D], F32)  # attention-weighted xn sums, PSUM-resident
        l_parts = lp.tile([HP, nsub], F32)  # per-subchunk softmax denominators

        with ExitStack() as sc_ctx:
            xp = sc_ctx.enter_context(tc.tile_pool(name="xp", bufs=3))
            sp = sc_ctx.enter_context(tc.tile_pool(name="sp", bufs=2))
            xnp = sc_ctx.enter_context(tc.tile_pool(name="xnp", bufs=2))
            xtp = sc_ctx.enter_context(tc.tile_pool(name="xtp", bufs=2))
            ksp = sc_ctx.enter_context(tc.tile_pool(name="ksp", bufs=2))
            rnp = sc_ctx.enter_context(tc.tile_pool(name="rnp", bufs=2))
            ep = sc_ctx.enter_context(tc.tile_pool(name="ep", bufs=2))
            etp = sc_ctx.enter_context(tc.tile_pool(name="etp", bufs=2))
            ktps = sc_ctx.enter_context(tc.tile_pool(name="ktps", bufs=2, space="PSUM"))
            scps = sc_ctx.enter_context(tc.tile_pool(name="scps", bufs=2, space="PSUM"))
            bnps = sc_ctx.enter_context(tc.tile_pool(name="bnps", bufs=1, space="PSUM"))
            trps = sc_ctx.enter_context(tc.tile_pool(name="trps", bufs=2, space="PSUM"))

            for s in range(nsub):
                x_sub = xp.tile([128, NBLK, D], F32)
                nc.sync.dma_start(x_sub, xr[:, s * NBLK : (s + 1) * NBLK, :])

                # --- LayerNorm stats ---
                stats = sp.tile([128, NBLK, 6], F32, tag="stats")
                for bb in range(NBLK):
                    nc.vector.bn_stats(stats[:, bb, :], x_sub[:, bb, :])
                mv = sp.tile([128, NBLK, 2], F32, tag="mv")
                for bb in range(NBLK):
                    nc.vector.bn_aggr(mv[:, bb, :], stats[:, bb, :])
                lnv = sp.tile([128, NBLK], F32, tag="lnv")
                nc.scalar.activation(lnv, mv[:, :, 1], AF.Ln, bias=eps_sb)
                rstd = sp.tile([128, NBLK], F32, tag="rstd")
                nc.scalar.activation(rstd, lnv, AF.Exp, scale=-0.5)
                nmr = sp.tile([128, NBLK], F32, tag="nmr")
                nc.vector.scalar_tensor_tensor(
                    nmr, mv[:, :, 0], -1.0, rstd, op0=OP.mult, op1=OP.mult
                )

                # --- normalize: xn0 = x*rstd + (-mu*rstd) ---
                xn0 = xnp.tile([128, NBLK, D], F32R)
                for bb in range(NBLK):
                    nc.gpsimd.tensor_scalar(
                        xn0[:, bb, :],
                        x_sub[:, bb, :],
                        rstd[:, bb : bb + 1],
                        nmr[:, bb : bb + 1],
                        op0=OP.mult,
                        op1=OP.add,
                    )

                # --- transpose xn0 -> xn_T [d-part, t-free] ---
                xn_T = xtp.tile([128, NDC, SUB], F32R)
                for bb in range(NBLK):
                    tr = trps.tile([128, NDC, 128], F32R, tag="tr")
                    for dc in range(NDC):
                        nc.tensor.matmul(
                            r(tr[:, dc, :]),
                            r(xn0[:, bb, dc * 128 : (dc + 1) * 128]),
                            r(ident_sb),
                            is_transpose=True,
                            start=(dc == 0),
                            stop=(dc == NDC - 1),
                        )
                    dst = xn_T[:, :, bb * 128 : (bb + 1) * 128]
                    if bb % 2 == 0:
                        nc.vector.tensor_copy(dst, tr)
                    else:
                        nc.scalar.copy(dst, tr)

                # --- K_T per head (PSUM) -> Ksq (bf16, SBUF) ---
                ksq = ksp.tile([128, H, SUB], BF16)
                for h in range(H):
                    kt = ktps.tile([128, SUB], F32, tag="kt")
                    for dc in range(NDC):
                        nc.tensor.matmul(
                            kt,
                            r(wkT_sb[:, dc, h * 128 : (h + 1) * 128]),
                            r(xn_T[:, dc, :]),
                            start=(dc == 0),
                            stop=(dc == NDC - 1),
                        )
                    nc.scalar.activation(
                        ksq[:, h, :], kt, AF.Square, bias=biask_sb[:, h : h + 1]
                    )

                # --- nrm2 reduced over partitions AND broadcast to [HP, SUB]
                # in one accumulating matmul with block-column ones masks ---
                bc_n2 = bnps.tile([HP, SUB], F32)
                for h in range(H):
                    nc.tensor.matmul(
                        bc_n2,
                        selk_sb[:, h, :],
                        ksq[:, h, :],
                        start=(h == 0),
                        stop=(h == H - 1),
                    )
                lnn = rnp.tile([HP, SUB], F32, tag="rnr")
                nc.scalar.activation(lnn, bc_n2, AF.Ln)
                rn = rnp.tile([HP, SUB], F32, tag="rn")
                nc.scalar.activation(rn, lnn, AF.Exp, scale=-0.5)

                # --- scores_T = Mq^T @ xn_T (PSUM) ---
                scp = scps.tile([HP, SUB], F32)
                for dc in range(NDC):
                    nc.tensor.matmul(
                        scp,
                        r(mq_sb[:, dc, :]),
                        r(xn_T[:, dc, :]),
                        start=(dc == 0),
                        stop=(dc == NDC - 1),
                    )

                # --- logits = (scores + sbias) * rnorm ; E = exp ---
                sE = ep.tile([HP, SUB], F32, tag="se")
                nc.vector.scalar_tensor_tensor(
                    sE, scp, sbias_sb, rn, op0=OP.add, op1=OP.mult
                )
                E = ep.tile([HP, SUB], F32R, tag="e")
                nc.scalar.activation(
                    E, sE, AF.Exp, accum_out=l_parts[:, s : s + 1]
                )

                # --- E_T and Y accumulation ---
                et_ps = trps.tile([128, NBLK, HP], F32R, tag="tr")
                for bb in range(NBLK):
                    nc.tensor.matmul(
                        r(et_ps[:, bb, :]),
                        r(E[:, bb * 128 : (bb + 1) * 128]),
                        r(ident_sb[:HP, :HP]),
                        is_transpose=True,
                        start=(bb == 0),
                        stop=(bb == NBLK - 1),
                    )
                ET = etp.tile([128, NBLK, HP], F32R)
                nc.vector.tensor_copy(ET, et_ps)
                for bb in range(NBLK):
                    nc.tensor.matmul(
                        y_ps,
                        r(ET[:, bb, :]),
                        r(xn0[:, bb, :]),
                        start=(s == 0 and bb == 0),
                        stop=(s == nsub - 1 and bb == NBLK - 1),
                    )

        # ---------------- final stage ----------------
        with ExitStack() as f_ctx:
            fp = f_ctx.enter_context(tc.tile_pool(name="fp", bufs=1))
            fps = f_ctx.enter_context(tc.tile_pool(name="fps", bufs=1, space="PSUM"))

            lsum = fp.tile([HP, 1], F32, tag="lsum")
            nc.vector.tensor_reduce(lsum, l_parts, axis=mybir.AxisListType.X, op=OP.add)
            linv = fp.tile([HP, 1], F32, tag="linv")
            nc.vector.reciprocal(linv, lsum)
            yn = fp.tile([HP, D], F32R, tag="yn")
            nc.vector.tensor_scalar(yn, y_ps, linv, None, op0=OP.mult)

            ynt_ps = fps.tile([128, NDC, HP], F32R, tag="f1")
            for dc in range(NDC):
                nc.tensor.matmul(
                    r(ynt_ps[:, dc, :]),
                    r(yn[:, dc * 128 : (dc + 1) * 128]),
                    r(ident_sb[:HP, :HP]),
                    is_transpose=True,
                    start=(dc == 0),
                    stop=(dc == NDC - 1),
                )
            ynt = fp.tile([128, NDC, HP], F32R, tag="ynt")
            nc.vector.tensor_copy(ynt, ynt_ps)

            # proto_tokens [P, D] = Yn @ WvT (per head) + bias_v
            proto_ps = fps.tile([P, D], F32, tag="f2")
            for h in range(H):
                for dc in range(NDC):
                    nc.tensor.matmul(
                        proto_ps[:, h * 128 : (h + 1) * 128],
                        r(ynt[:, dc, h * P : (h + 1) * P]),
                        r(wvT_sb[:, dc, h * 128 : (h + 1) * 128]),
                        start=(h == 0 and dc == 0),
                        stop=False,
                    )
            nc.tensor.matmul(proto_ps, r(ones24_sb), r(biasv_sb), start=False, stop=True)
            proto_sb = fp.tile([P, D], F32, tag="proto")
            nc.vector.tensor_copy(proto_sb, proto_ps)

            # proto_scores + rank-count top-k mask
            junk = fp.tile([P, D], F32, tag="junk")
            psc = fp.tile([P, 1], F32, tag="psc")
            nc.vector.scalar_tensor_tensor(
                junk, proto_sb, 1.0, proto_sb, op0=OP.mult, op1=OP.mult, accum_out=psc
            )
            pscT_ps = fps.tile([1, P], F32, tag="f3")
            nc.tensor.transpose(pscT_ps, psc, ident_sb[:P, :P].bitcast(F32))
            pscT = fp.tile([1, P], F32, tag="psct")
            nc.vector.tensor_copy(pscT, pscT_ps)
            g_ps = fps.tile([P, P], F32, tag="f4")
            nc.tensor.matmul(g_ps, ones24_sb.bitcast(F32), pscT, start=True, stop=True)
            junk2 = fp.tile([P, P], F32, tag="junk2")
            rank = fp.tile([P, 1], F32, tag="rank")
            nc.vector.scalar_tensor_tensor(
                junk2, g_ps, psc, onesPP_sb, op0=OP.is_gt, op1=OP.mult, accum_out=rank
            )
            mask = fp.tile([P, 1], F32, tag="mask")
            nc.vector.tensor_scalar(
                mask, rank, float(TOPK) - 0.5, None, op0=OP.is_lt
            )

            # z_T [128, NDC] = (proto^T @ mask) / TOPK
            z_ps = fps.tile([128, NDC], F32, tag="f5")
            for dc in range(NDC):
                nc.tensor.matmul(
                    z_ps[:, dc : dc + 1],
                    proto_sb[:, dc * 128 : (dc + 1) * 128],
                    mask,
                    start=(dc == 0),
                    stop=(dc == NDC - 1),
                )
            zT = fp.tile([128, NDC], F32, tag="zt")
            nc.scalar.activation(zT, z_ps, AF.Copy, scale=1.0 / TOPK)

            # h1_T [128, NDC] = W1 @ z  (columnwise)
            h1_ps = fps.tile([128, NDC], F32, tag="f6")
            for jc in range(NDC):
                for dc in range(NDC):
                    nc.tensor.matmul(
                        h1_ps[:, jc : jc + 1],
                        w1T_sb[:, dc, jc * 128 : (jc + 1) * 128],
                        zT[:, dc : dc + 1],
                        start=(jc == 0 and dc == 0),
                        stop=(jc == NDC - 1 and dc == NDC - 1),
                    )
            h1b = fp.tile([128, NDC], F32, tag="h1b")
            nc.vector.tensor_add(h1b, h1_ps, b1T_sb)
            h1g = fp.tile([128, NDC], F32, tag="h1g")
            nc.scalar.activation(h1g, h1b, AF.Sigmoid)
            h1s = fp.tile([128, NDC], F32, tag="h1s")
            nc.vector.tensor_mul(h1s, h1b, h1g)

            # out_row [1, D] = h1 @ W2^T + b2
            out_ps = fps.tile([1, D], F32, tag="f7")
            for dc in range(NDC):
                nc.tensor.matmul(
                    out_ps,
                    h1s[:, dc : dc + 1],
                    w2T_sb[:, dc, :],
                    start=(dc == 0),
                    stop=False,
                )
            nc.tensor.matmul(out_ps, ones1_sb.bitcast(F32), b2row_sb, start=False, stop=True)
            out_sb = fp.tile([1, D], F32, tag="out")
            nc.scalar.activation(out_sb, out_ps, AF.Copy)
            nc.sync.dma_start(y, out_sb)

    return nc


def make_host_weights(proto, ln_g, ln_b, Wq, Wk, Wv, W1, b1, W2, b2):
    f8 = np.float64
    g = ln_g.astype(f8)
    b = ln_b.astype(f8)
    Wk8, Wv8, Wq8 = Wk.astype(f8), Wv.astype(f8), Wq.astype(f8)
    Wk_eff = Wk8 * g[None, :]
    bias_k = Wk8 @ b
    Wv_eff = Wv8 * g[None, :]
    bias_v = Wv8 @ b
    Qp = proto.astype(f8) @ Wq8.T
    Qh = Qp.reshape(P, H, HD).transpose(1, 0, 2)
    nrm = np.maximum(np.linalg.norm(Qh, axis=-1, keepdims=True), 1e-12)
    Qh = Qh / nrm
    Mq = np.zeros((D, HP), f8)
    sbias = np.zeros((HP, 1), f8)
    for h in range(H):
        Mq[:, h * P : (h + 1) * P] = (Qh[h] @ Wk_eff[h * HD : (h + 1) * HD]).T / TEMP
        sbias[h * P : (h + 1) * P, 0] = Qh[h] @ bias_k[h * HD : (h + 1) * HD] / TEMP
    import ml_dtypes

    f = np.float32
    return {
        "wkT": np.ascontiguousarray(Wk_eff.T, f),
        "mq": np.ascontiguousarray(Mq, f),
        "sbias": sbias.astype(f),
        "biask": np.ascontiguousarray(bias_k.reshape(H, 128).T, f),
        "wvT": np.ascontiguousarray(Wv_eff.T, f),
        "biasv": np.ascontiguousarray(bias_v[None, :], f),
        "w1T": np.ascontiguousarray(W1.astype(f8).T, f),
        "b1T": np.ascontiguousarray(b1.astype(f8).reshape(NDC, 128).T, f),
        "w2T": np.ascontiguousarray(W2.astype(f8).T, f),
        "b2row": np.ascontiguousarray(b2.astype(f8)[None, :], f),
        "ident": np.eye(128, dtype=f),
        "selk": np.ascontiguousarray(
            np.broadcast_to(
                np.repeat(np.eye(H), P, axis=1)[None, :, :], (128, H, HP)
            ).astype(ml_dtypes.bfloat16)
        ),
        "ones24": np.ones((1, P), f),
        "ones1": np.ones((1, 1), f),
        "onesPP": np.ones((P, P), f),
    }


def _install_ntff_hook():
    """Recreate the missing antenv.axon_hooks registry and register the
    ctypes NTFF profile hook so run_bass_kernel_spmd(trace=True) works."""
    import types

    try:
        import antenv
    except ImportError:
        return False
    if "antenv.axon_hooks" not in sys.modules:
        m = types.ModuleType("antenv.axon_hooks")
        m._hook = None
        m.set_axon_ntff_profile_hook = lambda h: setattr(m, "_hook", h)
        m.get_axon_ntff_profile_hook = lambda: m._hook
        sys.modules["antenv.axon_hooks"] = m
        antenv.axon_hooks = m
    m = sys.modules["antenv.axon_hooks"]
    if m.get_axon_ntff_profile_hook() is None:
        try:
            from trn_agent_boot.trn_boot import _ntff_profile_via_ctypes

            hook = _ntff_profile_via_ctypes("/opt/axon/libaxon_pjrt.so")
            if hook is not None:
                m.set_axon_ntff_profile_hook(hook)
        except Exception as e:  # noqa: BLE001
            print("ntff hook install failed:", e)
            return False
    return m.get_axon_ntff_profile_hook() is not None


def kernel(
    x, proto, ln_g, ln_b, Wq, Wk, Wv, W1, b1, W2, b2, _trace=False, _t_total=T
):
    x = np.ascontiguousarray(np.asarray(x, np.float32))
    shared = make_host_weights(
        *(np.asarray(a, np.float32) for a in (proto, ln_g, ln_b, Wq, Wk, Wv, W1, b1, W2, b2))
    )
    nc = bacc.Bacc("TRN2", target_bir_lowering=False, debug=False, enable_asserts=False)
    build(nc, _t_total)
    nc.compile()
    in_maps = [dict(shared, xb=np.ascontiguousarray(x[bi])) for bi in range(B)]
    if _trace:
        _trace = _install_ntff_hook()
    res = run_bass_kernel_spmd(nc, in_maps, core_ids=list(range(B)), trace=_trace)
    out = np.stack([res.results[c]["y"][0] for c in range(B)])
    if _trace:
        kernel.last_results = res
    return out


# revision 22
# speedup vs baseline: 1.2382x; 1.1044x over previous
"""Trainium2 Bass kernel for ProductionTPA (sparse prototype attention).

Strategy (data-parallel over B, one batch element per NeuronCore):
  - LayerNorm stats via bn_stats in natural [t,d] layout; normalize with a
    single fused tensor_scalar (x*rstd + (-mu*rstd)).  ln_g/ln_b are folded
    into the weight matrices on the host (exact, O(D^2) work).
  - xn is transposed 128x128-blockwise on the PE so all D-contractions run
    on the tensor engine at full rate (float32r: fp32 storage, 1 cyc/row).
  - K is computed in transposed layout per head only to obtain per-token
    L2 norms: ACT squares K (PSUM->SBUF bf16), a ones-matmul reduces over
    the partition axis, giving nrm2 directly in [h, t] layout.
  - Scores are computed WITHOUT using K: the tiny matrix Mq = (Wk_h^T
    Qh_h)/TEMP is folded on the host, so scores_T = Mq^T @ xn_T.
  - Softmax over t needs no max subtraction: |logit| <= 1/TEMP by
    Cauchy-Schwarz (Q and K are L2-normalized), well within fp32 range.
    The denominator comes free via the activation accum_out.
  - The V projection is deferred: Y = E @ xn is accumulated in PSUM over
    the whole sequence, then projected once by Wv at the end (saves the
    full [T,D]x[D,D] V matmul).
  - top-k via rank counting (compare matrix -> rank -> mask), z = mask^T @
    proto_tokens / k, then a tiny fp32 MLP.
"""

import sys

sys.path.insert(0, "/opt/trn_rl_repo")

from contextlib import ExitStack

import numpy as np

import concourse.bass as bass
import concourse.tile as tile
from concourse import bacc
from concourse import mybir
from concourse.bass_utils import run_bass_kernel_spmd

def _patch_act_tables():
    """Steer the ACT table-set placement toward `natural_log_exp_and_others`
    (which holds ln+exp+square+copy — the whole hot loop) by hiding those
    functions from every other set in the map given to the placement pass.
    Set ids are positional, so order and count must be preserved."""
    import concourse.hw_specs as hw_specs
    import concourse.bacc as _bacc

    orig = hw_specs.get_activation_tables
    hot = {
        mybir.ActivationFunctionType.Ln,
        mybir.ActivationFunctionType.Exp,
        mybir.ActivationFunctionType.Square,
        mybir.ActivationFunctionType.Copy,
        mybir.ActivationFunctionType.Identity,
    }

    def patched(arch):
        tables = orig(arch)
        if "natural_log_exp_and_others" in tables:
            keep = tables["natural_log_exp_and_others"]
            for name, funcs in tables.items():
                if name != "natural_log_exp_and_others":
                    tables[name] = funcs - (hot & keep)
        return tables

    _bacc.get_activation_tables = patched


_patch_act_tables()

F32 = mybir.dt.float32
F32R = mybir.dt.float32r
BF16 = mybir.dt.bfloat16
AF = mybir.ActivationFunctionType
OP = mybir.AluOpType

B, T, D, H, P, HD = 8, 8192, 512, 4, 24, 128
HP = H * P  # 96
TEMP = 0.07
TOPK = 12
LN_EPS = 1e-5
SUB = 512  # tokens per subchunk
NBLK = SUB // 128  # 4
NDC = D // 128  # 4


def r(ap):
    return ap.bitcast(F32R)


def build(nc, t_total=T):
    nsub = t_total // SUB

    def din(name, shape, dt=F32):
        return nc.dram_tensor(name, shape, dt, kind="ExternalInput").ap()

    xb = din("xb", [t_total, D])
    wkT = din("wkT", [D, D], F32R)
    mq = din("mq", [D, HP], F32R)
    sbias = din("sbias", [HP, 1])
    biask = din("biask", [128, H])
    wvT = din("wvT", [D, D], F32R)
    biasv = din("biasv", [1, D], F32R)
    w1T = din("w1T", [D, D])
    b1T = din("b1T", [128, NDC])
    w2T = din("w2T", [D, D])
    b2row = din("b2row", [1, D])
    ident = din("ident", [128, 128], F32R)
    selk = din("selk", [128, H, HP], BF16)
    ones24 = din("ones24", [1, P], F32R)
    ones1 = din("ones1", [1, 1], F32R)
    onesPP = din("onesPP", [P, P])
    y = nc.dram_tensor("y", [1, D], F32, kind="ExternalOutput").ap()

    xr = xb.rearrange("(n p) d -> p n d", p=128)  # [128, t/128, D]

    with tile.TileContext(nc) as tc, ExitStack() as ctx:
        wp = ctx.enter_context(tc.tile_pool(name="wp", bufs=1))
        yp = ctx.enter_context(tc.tile_pool(name="yp", bufs=1, space="PSUM"))
        lp = ctx.enter_context(tc.tile_pool(name="lp", bufs=1))

        # resident weights
        wkT_sb = wp.tile([128, NDC, D], F32R)
        nc.sync.dma_start(wkT_sb, wkT.rearrange("(c p) j -> p c j", p=128))
        mq_sb = wp.tile([128, NDC, HP], F32R)
        nc.sync.dma_start(mq_sb, mq.rearrange("(c p) j -> p c j", p=128))
        wvT_sb = wp.tile([128, NDC, D], F32R)
        nc.sync.dma_start(wvT_sb, wvT.rearrange("(c p) j -> p c j", p=128))
        w1T_sb = wp.tile([128, NDC, D], F32)
        nc.sync.dma_start(w1T_sb, w1T.rearrange("(c p) j -> p c j", p=128))
        w2T_sb = wp.tile([128, NDC, D], F32)
        nc.sync.dma_start(w2T_sb, w2T.rearrange("(c p) j -> p c j", p=128))
        ident_sb = wp.tile([128, 128], F32R)
        nc.sync.dma_start(ident_sb, ident)
        selk_sb = wp.tile([128, H, HP], BF16)
        nc.sync.dma_start(selk_sb, selk)
        sbias_sb = wp.tile([HP, 1], F32)
        nc.sync.dma_start(sbias_sb, sbias)
        biask_sb = wp.tile([128, H], F32)
        nc.sync.dma_start(biask_sb, biask)
        biasv_sb = wp.tile([1, D], F32R)
        nc.sync.dma_start(biasv_sb, biasv)
        b1T_sb = wp.tile([128, NDC], F32)
        nc.sync.dma_start(b1T_sb, b1T)
        b2row_sb = wp.tile([1, D], F32)
        nc.sync.dma_start(b2row_sb, b2row)
        ones24_sb = wp.tile([1, P], F32R)
        nc.sync.dma_start(ones24_sb, ones24)
        ones1_sb = wp.tile([1, 1], F32R)
        nc.sync.dma_start(ones1_sb, ones1)
        onesPP_sb = wp.tile([P, P], F32)
        nc.sync.dma_start(onesPP_sb, onesPP)

        eps_sb = wp.tile([128, 1], F32)
        nc.vector.memset(eps_sb, LN_EPS)

        y_ps = yp.tile([HP, D], F32)  # attention-weighted xn sums, PSUM-resident
        l_parts = lp.tile([HP, nsub], F32)  # per-subchunk softmax denominators

        with ExitStack() as sc_ctx:
            xp = sc_ctx.enter_context(tc.tile_pool(name="xp", bufs=3))
            sp = sc_ctx.enter_context(tc.tile_pool(name="sp", bufs=2))
            xnp = sc_ctx.enter_context(tc.tile_pool(name="xnp", bufs=2))
            xtp = sc_ctx.enter_context(tc.tile_pool(name="xtp", bufs=2))
            ksp = sc_ctx.enter_context(tc.tile_pool(name="ksp", bufs=2))
            rnp = sc_ctx.enter_context(tc.tile_pool(name="rnp", bufs=2))
            ep = sc_ctx.enter_context(tc.tile_pool(name="ep", bufs=2))
            etp = sc_ctx.enter_context(tc.tile_pool(name="etp", bufs=2))
            ktps = sc_ctx.enter_context(tc.tile_pool(name="ktps", bufs=2, space="PSUM"))
            scps = sc_ctx.enter_context(tc.tile_pool(name="scps", bufs=2, space="PSUM"))
            bnps = sc_ctx.enter_context(tc.tile_pool(name="bnps", bufs=1, space="PSUM"))
            trps = sc_ctx.enter_context(tc.tile_pool(name="trps", bufs=2, space="PSUM"))

            for s in range(nsub):
                x_sub = xp.tile([128, NBLK, D], F32)
                nc.sync.dma_start(x_sub, xr[:, s * NBLK : (s + 1) * NBLK, :])

                # --- LayerNorm stats ---
                stats = sp.tile([128, NBLK, 6], F32, tag="stats")
                for bb in range(NBLK):
                    nc.vector.bn_stats(stats[:, bb, :], x_sub[:, bb, :])
                mv = sp.tile([128, NBLK, 2], F32, tag="mv")
                for bb in range(NBLK):
                    nc.vector.bn_aggr(mv[:, bb, :], stats[:, bb, :])
                lnv = sp.tile([128, NBLK], F32, tag="lnv")
                nc.scalar.activation(lnv, mv[:, :, 1], AF.Ln, bias=eps_sb)
                rstd = sp.tile([128, NBLK], F32, tag="rstd")
                nc.scalar.activation(rstd, lnv, AF.Exp, scale=-0.5)
                nmr = sp.tile([128, NBLK], F32, tag="nmr")
                nc.vector.scalar_tensor_tensor(
                    nmr, mv[:, :, 0], -1.0, rstd, op0=OP.mult, op1=OP.mult
                )

                # --- normalize: xn0 = x*rstd + (-mu*rstd) ---
                xn0 = xnp.tile([128, NBLK, D], F32R)
                for bb in range(NBLK):
                    nc.gpsimd.tensor_scalar(
                        xn0[:, bb, :],
                        x_sub[:, bb, :],
                        rstd[:, bb : bb + 1],
                        nmr[:, bb : bb + 1],
                        op0=OP.mult,
                        op1=OP.add,
                    )

                # --- transpose xn0 -> xn_T [d-part, t-free] ---
                xn_T = xtp.tile([128, NDC, SUB], F32R)
                for bb in range(NBLK):
                    tr = trps.tile([128, NDC, 128], F32R, tag="tr")
                    for dc in range(NDC):
                        nc.tensor.matmul(
                            r(tr[:, dc, :]),
                            r(xn0[:, bb, dc * 128 : (dc + 1) * 128]),
                            r(ident_sb),
                            is_transpose=True,
                            start=(dc == 0),
                            stop=(dc == NDC - 1),
                        )
                    dst = xn_T[:, :, bb * 128 : (bb + 1) * 128]
                    if bb % 2 == 0:
                        nc.vector.tensor_copy(dst, tr)
                    else:
                        nc.scalar.copy(dst, tr)

                # --- K_T per head (PSUM) -> Ksq (bf16, SBUF) ---
                ksq = ksp.tile([128, H, SUB], BF16)
                for h in range(H):
                    kt = ktps.tile([128, SUB], F32, tag="kt")
                    for dc in range(NDC):
                        nc.tensor.matmul(
                            kt,
                            r(wkT_sb[:, dc, h * 128 : (h + 1) * 128]),
                            r(xn_T[:, dc, :]),
                            start=(dc == 0),
                            stop=(dc == NDC - 1),
                        )
                    nc.scalar.activation(
                        ksq[:, h, :], kt, AF.Square, bias=biask_sb[:, h : h + 1]
                    )

                # --- nrm2 reduced over partitions AND broadcast to [HP, SUB]
                # in one accumulating matmul with block-column ones masks ---
                bc_n2 = bnps.tile([HP, SUB], F32)
                for h in range(H):
                    nc.tensor.matmul(
                        bc_n2,
                        selk_sb[:, h, :],
                        ksq[:, h, :],
                        start=(h == 0),
                        stop=(h == H - 1),
                    )
                lnn = rnp.tile([HP, SUB], F32, tag="rnr")
                nc.scalar.activation(lnn, bc_n2, AF.Ln)
                rn = rnp.tile([HP, SUB], F32, tag="rn")
                nc.scalar.activation(rn, lnn, AF.Exp, scale=-0.5)

                # --- scores_T = Mq^T @ xn_T (PSUM) ---
                scp = scps.tile([HP, SUB], F32)
                for dc in range(NDC):
                    nc.tensor.matmul(
                        scp,
                        r(mq_sb[:, dc, :]),
                        r(xn_T[:, dc, :]),
                        start=(dc == 0),
                        stop=(dc == NDC - 1),
                    )

                # --- logits = (scores + sbias) * rnorm ; E = exp ---
                sE = ep.tile([HP, SUB], F32, tag="se")
                nc.vector.scalar_tensor_tensor(
                    sE, scp, sbias_sb, rn, op0=OP.add, op1=OP.mult
                )
                E = ep.tile([HP, SUB], F32R, tag="e")
                nc.scalar.activation(
                    E, sE, AF.Exp, accum_out=l_parts[:, s : s + 1]
                )

                # --- E_T and Y accumulation ---
                et_ps = trps.tile([128, NBLK, HP], F32R, tag="tr")
                for bb in range(NBLK):
                    nc.tensor.matmul(
                        r(et_ps[:, bb, :]),
                        r(E[:, bb * 128 : (bb + 1) * 128]),
                        r(ident_sb[:HP, :HP]),
                        is_transpose=True,
                        start=(bb == 0),
                        stop=(bb == NBLK - 1),
                    )
                ET = etp.tile([128, NBLK, HP], F32R)
                nc.vector.tensor_copy(ET, et_ps)
                for bb in range(NBLK):
                    nc.tensor.matmul(
                        y_ps,
                        r(ET[:, bb, :]),
                        r(xn0[:, bb, :]),
                        start=(s == 0 and bb == 0),
                        stop=(s == nsub - 1 and bb == NBLK - 1),
                    )

        # ---------------- final stage ----------------
        with ExitStack() as f_ctx:
            fp = f_ctx.enter_context(tc.tile_pool(name="fp", bufs=1))
            fps = f_ctx.enter_context(tc.tile_pool(name="fps", bufs=1, space="PSUM"))

            lsum = fp.tile([HP, 1], F32, tag="lsum")
            nc.vector.tensor_reduce(lsum, l_parts, axis=mybir.AxisListType.X, op=OP.add)
            linv = fp.tile([HP, 1], F32, tag="linv")
            nc.vector.reciprocal(linv, lsum)
            yn = fp.tile([HP, D], F32R, tag="yn")
            nc.vector.tensor_scalar(yn, y_ps, linv, None, op0=OP.mult)

            ynt_ps = fps.tile([128, NDC, HP], F32R, tag="f1")
            for dc in range(NDC):
                nc.tensor.matmul(
                    r(ynt_ps[:, dc, :]),
                    r(yn[:, dc * 128 : (dc + 1) * 128]),
                    r(ident_sb[:HP, :HP]),
                    is_transpose=True,
                    start=(dc == 0),
                    stop=(dc == NDC - 1),
                )
            ynt = fp.tile([128, NDC, HP], F32R, tag="ynt")
            nc.vector.tensor_copy(ynt, ynt_ps)

            # proto_tokens [P, D] = Yn @ WvT (per head) + bias_v
            proto_ps = fps.tile([P, D], F32, tag="f2")
            for h in range(H):
                for dc in range(NDC):
                    nc.tensor.matmul(
                        proto_ps[:, h * 128 : (h + 1) * 128],
                        r(ynt[:, dc, h * P : (h + 1) * P]),
                        r(wvT_sb[:, dc, h * 128 : (h + 1) * 128]),
                        start=(h == 0 and dc == 0),
                        stop=False,
                    )
            nc.tensor.matmul(proto_ps, r(ones24_sb), r(biasv_sb), start=False, stop=True)
            proto_sb = fp.tile([P, D], F32, tag="proto")
            nc.vector.tensor_copy(proto_sb, proto_ps)

            # proto_scores + rank-count top-k mask
            junk = fp.tile([P, D], F32, tag="junk")
            psc = fp.tile([P, 1], F32, tag="psc")
            nc.vector.scalar_tensor_tensor(
                junk, proto_sb, 1.0, proto_sb, op0=OP.mult, op1=OP.mult, accum_out=psc
            )
            pscT_ps = fps.tile([1, P], F32, tag="f3")
            nc.tensor.transpose(pscT_ps, psc, ident_sb[:P, :P].bitcast(F32))
            pscT = fp.tile([1, P], F32, tag="psct")
            nc.vector.tensor_copy(pscT, pscT_ps)
            g_ps = fps.tile([P, P], F32, tag="f4")
            nc.tensor.matmul(g_ps, ones24_sb.bitcast(F32), pscT, start=True, stop=True)
            junk2 = fp.tile([P, P], F32, tag="junk2")
            rank = fp.tile([P, 1], F32, tag="rank")
            nc.vector.scalar_tensor_tensor(
                junk2, g_ps, psc, onesPP_sb, op0=OP.is_gt, op1=OP.mult, accum_out=rank
            )
            mask = fp.tile([P, 1], F32, tag="mask")
            nc.vector.tensor_scalar(
                mask, rank, float(TOPK) - 0.5, None, op0=OP.is_lt
            )

            # z_T [128, NDC] = (proto^T @ mask) / TOPK
            z_ps = fps.tile([128, NDC], F32, tag="f5")
            for dc in range(NDC):
                nc.tensor.matmul(
                    z_ps[:, dc : dc + 1],
                    proto_sb[:, dc * 128 : (dc + 1) * 128],
                    mask,
                    start=(dc == 0),
                    stop=(dc == NDC - 1),
                )
            zT = fp.tile([128, NDC], F32, tag="zt")
            nc.scalar.activation(zT, z_ps, AF.Copy, scale=1.0 / TOPK)

            # h1_T [128, NDC] = W1 @ z  (columnwise)
            h1_ps = fps.tile([128, NDC], F32, tag="f6")
            for jc in range(NDC):
                for dc in range(NDC):
                    nc.tensor.matmul(
                        h1_ps[:, jc : jc + 1],
                        w1T_sb[:, dc, jc * 128 : (jc + 1) * 128],
                        zT[:, dc : dc + 1],
                        start=(jc == 0 and dc == 0),
                        stop=(jc == NDC - 1 and dc == NDC - 1),
                    )
            h1b = fp.tile([128, NDC], F32, tag="h1b")
            nc.vector.tensor_add(h1b, h1_ps, b1T_sb)
            h1g = fp.tile([128, NDC], F32, tag="h1g")
            nc.scalar.activation(h1g, h1b, AF.Sigmoid)
            h1s = fp.tile([128, NDC], F32, tag="h1s")
            nc.vector.tensor_mul(h1s, h1b, h1g)

            # out_row [1, D] = h1 @ W2^T + b2
            out_ps = fps.tile([1, D], F32, tag="f7")
            for dc in range(NDC):
                nc.tensor.matmul(
                    out_ps,
                    h1s[:, dc : dc + 1],
                    w2T_sb[:, dc, :],
                    start=(dc == 0),
                    stop=False,
                )
            nc.tensor.matmul(out_ps, ones1_sb.bitcast(F32), b2row_sb, start=False, stop=True)
            out_sb = fp.tile([1, D], F32, tag="out")
            nc.scalar.activation(out_sb, out_ps, AF.Copy)
            nc.sync.dma_start(y, out_sb)

    return nc


def make_host_weights(proto, ln_g, ln_b, Wq, Wk, Wv, W1, b1, W2, b2):
    f8 = np.float64
    g = ln_g.astype(f8)
    b = ln_b.astype(f8)
    Wk8, Wv8, Wq8 = Wk.astype(f8), Wv.astype(f8), Wq.astype(f8)
    Wk_eff = Wk8 * g[None, :]
    bias_k = Wk8 @ b
    Wv_eff = Wv8 * g[None, :]
    bias_v = Wv8 @ b
    Qp = proto.astype(f8) @ Wq8.T
    Qh = Qp.reshape(P, H, HD).transpose(1, 0, 2)
    nrm = np.maximum(np.linalg.norm(Qh, axis=-1, keepdims=True), 1e-12)
    Qh = Qh / nrm
    Mq = np.zeros((D, HP), f8)
    sbias = np.zeros((HP, 1), f8)
    for h in range(H):
        Mq[:, h * P : (h + 1) * P] = (Qh[h] @ Wk_eff[h * HD : (h + 1) * HD]).T / TEMP
        sbias[h * P : (h + 1) * P, 0] = Qh[h] @ bias_k[h * HD : (h + 1) * HD] / TEMP
    import ml_dtypes

    f = np.float32
    return {
        "wkT": np.ascontiguousarray(Wk_eff.T, f),
        "mq": np.ascontiguousarray(Mq, f),
        "sbias": sbias.astype(f),
        "biask": np.ascontiguousarray(bias_k.reshape(H, 128).T, f),
        "wvT": np.ascontiguousarray(Wv_eff.T, f),
        "biasv": np.ascontiguousarray(bias_v[None, :], f),
        "w1T": np.ascontiguousarray(W1.astype(f8).T, f),
        "b1T": np.ascontiguousarray(b1.astype(f8).reshape(NDC, 128).T, f),
        "w2T": np.ascontiguousarray(W2.astype(f8).T, f),
        "b2row": np.ascontiguousarray(b2.astype(f8)[None, :], f),
        "ident": np.eye(128, dtype=f),
        "selk": np.ascontiguousarray(
            np.broadcast_to(
                np.repeat(np.eye(H), P, axis=1)[None, :, :], (128, H, HP)
            ).astype(ml_dtypes.bfloat16)
        ),
        "ones24": np.ones((1, P), f),
        "ones1": np.ones((1, 1), f),
        "onesPP": np.ones((P, P), f),
    }


def _install_ntff_hook():
    """Recreate the missing antenv.axon_hooks registry and register the
    ctypes NTFF profile hook so run_bass_kernel_spmd(trace=True) works."""
    import types

    try:
        import antenv
    except ImportError:
        return False
    if "antenv.axon_hooks" not in sys.modules:
        m = types.ModuleType("antenv.axon_hooks")
        m._hook = None
        m.set_axon_ntff_profile_hook = lambda h: setattr(m, "_hook", h)
        m.get_axon_ntff_profile_hook = lambda: m._hook
        sys.modules["antenv.axon_hooks"] = m
        antenv.axon_hooks = m
    m = sys.modules["antenv.axon_hooks"]
    if m.get_axon_ntff_profile_hook() is None:
        try:
            from trn_agent_boot.trn_boot import _ntff_profile_via_ctypes

            hook = _ntff_profile_via_ctypes("/opt/axon/libaxon_pjrt.so")
            if hook is not None:
                m.set_axon_ntff_profile_hook(hook)
        except Exception as e:  # noqa: BLE001
            print("ntff hook install failed:", e)
            return False
    return m.get_axon_ntff_profile_hook() is not None


def kernel(
    x, proto, ln_g, ln_b, Wq, Wk, Wv, W1, b1, W2, b2, _trace=False, _t_total=T
):
    x = np.ascontiguousarray(np.asarray(x, np.float32))
    shared = make_host_weights(
        *(np.asarray(a, np.float32) for a in (proto, ln_g, ln_b, Wq, Wk, Wv, W1, b1, W2, b2))
    )
    nc = bacc.Bacc("TRN2", target_bir_lowering=False, debug=False, enable_asserts=False)
    build(nc, _t_total)
    nc.compile()
    in_maps = [dict(shared, xb=np.ascontiguousarray(x[bi])) for bi in range(B)]
    if _trace:
        _trace = _install_ntff_hook()
    res = run_bass_kernel_spmd(nc, in_maps, core_ids=list(range(B)), trace=_trace)
    out = np.stack([res.results[c]["y"][0] for c in range(B)])
    if _trace:
        kernel.last_results = res
    return out


# revision 23
# speedup vs baseline: 1.5463x; 1.2488x over previous
"""Trainium2 Bass kernel for ProductionTPA (sparse prototype attention).

Strategy (data-parallel over B, one batch element per NeuronCore):
  - LayerNorm stats via bn_stats in natural [t,d] layout; normalize with a
    single fused tensor_scalar (x*rstd + (-mu*rstd)).  ln_g/ln_b are folded
    into the weight matrices on the host (exact, O(D^2) work).
  - xn is transposed 128x128-blockwise on the PE so all D-contractions run
    on the tensor engine at full rate (float32r: fp32 storage, 1 cyc/row).
  - K is computed in transposed layout per head only to obtain per-token
    L2 norms: ACT squares K (PSUM->SBUF bf16), a ones-matmul reduces over
    the partition axis, giving nrm2 directly in [h, t] layout.
  - Scores are computed WITHOUT using K: the tiny matrix Mq = (Wk_h^T
    Qh_h)/TEMP is folded on the host, so scores_T = Mq^T @ xn_T.
  - Softmax over t needs no max subtraction: |logit| <= 1/TEMP by
    Cauchy-Schwarz (Q and K are L2-normalized), well within fp32 range.
    The denominator comes free via the activation accum_out.
  - The V projection is deferred: Y = E @ xn is accumulated in PSUM over
    the whole sequence, then projected once by Wv at the end (saves the
    full [T,D]x[D,D] V matmul).
  - top-k via rank counting (compare matrix -> rank -> mask), z = mask^T @
    proto_tokens / k, then a tiny fp32 MLP.
"""

import sys

sys.path.insert(0, "/opt/trn_rl_repo")

from contextlib import ExitStack

import numpy as np

import concourse.bass as bass
import concourse.tile as tile
from concourse import bacc
from concourse import mybir
from concourse.bass_utils import run_bass_kernel_spmd

def _patch_act_tables():
    """Steer the ACT table-set placement toward `natural_log_exp_and_others`
    (which holds ln+exp+square+copy — the whole hot loop) by hiding those
    functions from every other set in the map given to the placement pass.
    Set ids are positional, so order and count must be preserved."""
    import concourse.hw_specs as hw_specs
    import concourse.bacc as _bacc

    orig = hw_specs.get_activation_tables
    hot = {
        mybir.ActivationFunctionType.Ln,
        mybir.ActivationFunctionType.Exp,
        mybir.ActivationFunctionType.Square,
        mybir.ActivationFunctionType.Copy,
        mybir.ActivationFunctionType.Identity,
    }

    def patched(arch):
        tables = orig(arch)
        if "natural_log_exp_and_others" in tables:
            keep = tables["natural_log_exp_and_others"]
            for name, funcs in tables.items():
                if name != "natural_log_exp_and_others":
                    tables[name] = funcs - (hot & keep)
        return tables

    _bacc.get_activation_tables = patched


_patch_act_tables()

F32 = mybir.dt.float32
F32R = mybir.dt.float32r
BF16 = mybir.dt.bfloat16
AF = mybir.ActivationFunctionType
OP = mybir.AluOpType

B, T, D, H, P, HD = 8, 8192, 512, 4, 24, 128
HP = H * P  # 96
TEMP = 0.07
TOPK = 12
LN_EPS = 1e-5
SUB = 512  # tokens per subchunk
NBLK = SUB // 128  # 4
NDC = D // 128  # 4


def r(ap):
    return ap.bitcast(F32R)


def build(nc, t_total=T):
    nsub = t_total // SUB

    def din(name, shape, dt=F32):
        return nc.dram_tensor(name, shape, dt, kind="ExternalInput").ap()

    xb = din("xb", [t_total, D])
    wkT = din("wkT", [D, D], F32R)
    mq = din("mq", [D, HP], F32R)
    sbias = din("sbias", [HP, 1])
    biask = din("biask", [128, H])
    wvT = din("wvT", [D, D], F32R)
    biasv = din("biasv", [1, D], F32R)
    w1T = din("w1T", [D, D])
    b1T = din("b1T", [128, NDC])
    w2T = din("w2T", [D, D])
    b2row = din("b2row", [1, D])
    ident = din("ident", [128, 128], F32R)
    selk = din("selk", [128, H, HP], BF16)
    ones24 = din("ones24", [1, P], F32R)
    ones1 = din("ones1", [1, 1], F32R)
    onesPP = din("onesPP", [P, P])
    y = nc.dram_tensor("y", [1, D], F32, kind="ExternalOutput").ap()

    xr = xb.rearrange("(n p) d -> p n d", p=128)  # [128, t/128, D]

    with tile.TileContext(nc) as tc, ExitStack() as ctx:
        wp = ctx.enter_context(tc.tile_pool(name="wp", bufs=1))
        yp = ctx.enter_context(tc.tile_pool(name="yp", bufs=1, space="PSUM"))
        lp = ctx.enter_context(tc.tile_pool(name="lp", bufs=1))

        # resident weights
        wkT_sb = wp.tile([128, NDC, D], F32R)
        nc.sync.dma_start(wkT_sb, wkT.rearrange("(c p) j -> p c j", p=128))
        mq_sb = wp.tile([128, NDC, HP], F32R)
        nc.sync.dma_start(mq_sb, mq.rearrange("(c p) j -> p c j", p=128))
        wvT_sb = wp.tile([128, NDC, D], F32R)
        nc.sync.dma_start(wvT_sb, wvT.rearrange("(c p) j -> p c j", p=128))
        w1T_sb = wp.tile([128, NDC, D], F32)
        nc.sync.dma_start(w1T_sb, w1T.rearrange("(c p) j -> p c j", p=128))
        w2T_sb = wp.tile([128, NDC, D], F32)
        nc.sync.dma_start(w2T_sb, w2T.rearrange("(c p) j -> p c j", p=128))
        ident_sb = wp.tile([128, 128], F32R)
        nc.sync.dma_start(ident_sb, ident)
        selk_sb = wp.tile([128, H, HP], BF16)
        nc.sync.dma_start(selk_sb, selk)
        sbias_sb = wp.tile([HP, 1], F32)
        nc.sync.dma_start(sbias_sb, sbias)
        biask_sb = wp.tile([128, H], F32)
        nc.sync.dma_start(biask_sb, biask)
        biasv_sb = wp.tile([1, D], F32R)
        nc.sync.dma_start(biasv_sb, biasv)
        b1T_sb = wp.tile([128, NDC], F32)
        nc.sync.dma_start(b1T_sb, b1T)
        b2row_sb = wp.tile([1, D], F32)
        nc.sync.dma_start(b2row_sb, b2row)
        ones24_sb = wp.tile([1, P], F32R)
        nc.sync.dma_start(ones24_sb, ones24)
        ones1_sb = wp.tile([1, 1], F32R)
        nc.sync.dma_start(ones1_sb, ones1)
        onesPP_sb = wp.tile([P, P], F32)
        nc.sync.dma_start(onesPP_sb, onesPP)

        eps_sb = wp.tile([128, 1], F32)
        nc.vector.memset(eps_sb, LN_EPS)

        y_ps = yp.tile([HP, D], F32)  # attention-weighted xn sums, PSUM-resident
        l_parts = lp.tile([HP, nsub], F32)  # per-subchunk softmax denominators

        with ExitStack() as sc_ctx:
            xp = sc_ctx.enter_context(tc.tile_pool(name="xp", bufs=3))
            sp = sc_ctx.enter_context(tc.tile_pool(name="sp", bufs=2))
            xnp = sc_ctx.enter_context(tc.tile_pool(name="xnp", bufs=3))
            xtp = sc_ctx.enter_context(tc.tile_pool(name="xtp", bufs=2))
            ksp = sc_ctx.enter_context(tc.tile_pool(name="ksp", bufs=2))
            rnp = sc_ctx.enter_context(tc.tile_pool(name="rnp", bufs=2))
            ep = sc_ctx.enter_context(tc.tile_pool(name="ep", bufs=3))
            etp = sc_ctx.enter_context(tc.tile_pool(name="etp", bufs=2))
            ktps = sc_ctx.enter_context(tc.tile_pool(name="ktps", bufs=2, space="PSUM"))
            scps = sc_ctx.enter_context(tc.tile_pool(name="scps", bufs=2, space="PSUM"))
            bnps = sc_ctx.enter_context(tc.tile_pool(name="bnps", bufs=1, space="PSUM"))
            trps = sc_ctx.enter_context(tc.tile_pool(name="trps", bufs=2, space="PSUM"))

            pending = []

            def emit_stage_b(s, E, xn0):
                # --- E_T and Y accumulation ---
                et_ps = trps.tile([128, NBLK, HP], F32R, tag="tr")
                for bb in range(NBLK):
                    nc.tensor.matmul(
                        r(et_ps[:, bb, :]),
                        r(E[:, bb * 128 : (bb + 1) * 128]),
                        r(ident_sb[:HP, :HP]),
                        is_transpose=True,
                        start=(bb == 0),
                        stop=(bb == NBLK - 1),
                    )
                ET = etp.tile([128, NBLK, HP], F32R)
                nc.vector.tensor_copy(ET, et_ps)
                for bb in range(NBLK):
                    nc.tensor.matmul(
                        y_ps,
                        r(ET[:, bb, :]),
                        r(xn0[:, bb, :]),
                        start=(s == 0 and bb == 0),
                        stop=(s == nsub - 1 and bb == NBLK - 1),
                    )

            for s in range(nsub):
                x_sub = xp.tile([128, NBLK, D], F32)
                nc.sync.dma_start(x_sub, xr[:, s * NBLK : (s + 1) * NBLK, :])

                # --- LayerNorm stats ---
                stats = sp.tile([128, NBLK, 6], F32, tag="stats")
                for bb in range(NBLK):
                    nc.vector.bn_stats(stats[:, bb, :], x_sub[:, bb, :])
                mv = sp.tile([128, NBLK, 2], F32, tag="mv")
                for bb in range(NBLK):
                    nc.vector.bn_aggr(mv[:, bb, :], stats[:, bb, :])
                lnv = sp.tile([128, NBLK], F32, tag="lnv")
                nc.scalar.activation(lnv, mv[:, :, 1], AF.Ln, bias=eps_sb)
                rstd = sp.tile([128, NBLK], F32, tag="rstd")
                nc.scalar.activation(rstd, lnv, AF.Exp, scale=-0.5)
                nmr = sp.tile([128, NBLK], F32, tag="nmr")
                nc.vector.scalar_tensor_tensor(
                    nmr, mv[:, :, 0], -1.0, rstd, op0=OP.mult, op1=OP.mult
                )

                # --- normalize: xn0 = x*rstd + (-mu*rstd) ---
                xn0 = xnp.tile([128, NBLK, D], F32R)
                for bb in range(NBLK):
                    nc.gpsimd.tensor_scalar(
                        xn0[:, bb, :],
                        x_sub[:, bb, :],
                        rstd[:, bb : bb + 1],
                        nmr[:, bb : bb + 1],
                        op0=OP.mult,
                        op1=OP.add,
                    )

                # --- transpose xn0 -> xn_T [d-part, t-free] ---
                xn_T = xtp.tile([128, NDC, SUB], F32R)
                for bb in range(NBLK):
                    tr = trps.tile([128, NDC, 128], F32R, tag="tr")
                    for dc in range(NDC):
                        nc.tensor.matmul(
                            r(tr[:, dc, :]),
                            r(xn0[:, bb, dc * 128 : (dc + 1) * 128]),
                            r(ident_sb),
                            is_transpose=True,
                            start=(dc == 0),
                            stop=(dc == NDC - 1),
                        )
                    dst = xn_T[:, :, bb * 128 : (bb + 1) * 128]
                    if bb % 2 == 0:
                        nc.vector.tensor_copy(dst, tr)
                    else:
                        nc.scalar.copy(dst, tr)

                # --- K_T per head (PSUM) -> Ksq (bf16, SBUF) ---
                ksq = ksp.tile([128, H, SUB], BF16)
                for h in range(H):
                    kt = ktps.tile([128, SUB], F32, tag="kt")
                    for dc in range(NDC):
                        nc.tensor.matmul(
                            kt,
                            r(wkT_sb[:, dc, h * 128 : (h + 1) * 128]),
                            r(xn_T[:, dc, :]),
                            start=(dc == 0),
                            stop=(dc == NDC - 1),
                        )
                    nc.scalar.activation(
                        ksq[:, h, :], kt, AF.Square, bias=biask_sb[:, h : h + 1]
                    )

                # --- nrm2 reduced over partitions AND broadcast to [HP, SUB]
                # in one accumulating matmul with block-column ones masks ---
                bc_n2 = bnps.tile([HP, SUB], F32)
                for h in range(H):
                    nc.tensor.matmul(
                        bc_n2,
                        selk_sb[:, h, :],
                        ksq[:, h, :],
                        start=(h == 0),
                        stop=(h == H - 1),
                    )
                lnn = rnp.tile([HP, SUB], F32, tag="rnr")
                nc.scalar.activation(lnn, bc_n2, AF.Ln)
                rn = rnp.tile([HP, SUB], F32, tag="rn")
                nc.scalar.activation(rn, lnn, AF.Exp, scale=-0.5)

                # --- scores_T = Mq^T @ xn_T (PSUM) ---
                scp = scps.tile([HP, SUB], F32)
                for dc in range(NDC):
                    nc.tensor.matmul(
                        scp,
                        r(mq_sb[:, dc, :]),
                        r(xn_T[:, dc, :]),
                        start=(dc == 0),
                        stop=(dc == NDC - 1),
                    )

                # --- logits = (scores + sbias) * rnorm ; E = exp ---
                sE = ep.tile([HP, SUB], F32, tag="se")
                nc.vector.scalar_tensor_tensor(
                    sE, scp, sbias_sb, rn, op0=OP.add, op1=OP.mult
                )
                E = ep.tile([HP, SUB], F32R, tag="e")
                nc.scalar.activation(
                    E, sE, AF.Exp, accum_out=l_parts[:, s : s + 1]
                )
                pending.append((s, E, xn0))
                if s >= 1:
                    emit_stage_b(*pending.pop(0))
            while pending:
                emit_stage_b(*pending.pop(0))

        # ---------------- final stage ----------------
        with ExitStack() as f_ctx:
            fp = f_ctx.enter_context(tc.tile_pool(name="fp", bufs=1))
            fps = f_ctx.enter_context(tc.tile_pool(name="fps", bufs=1, space="PSUM"))

            lsum = fp.tile([HP, 1], F32, tag="lsum")
            nc.vector.tensor_reduce(lsum, l_parts, axis=mybir.AxisListType.X, op=OP.add)
            linv = fp.tile([HP, 1], F32, tag="linv")
            nc.vector.reciprocal(linv, lsum)
            yn = fp.tile([HP, D], F32R, tag="yn")
            nc.vector.tensor_scalar(yn, y_ps, linv, None, op0=OP.mult)

            ynt_ps = fps.tile([128, NDC, HP], F32R, tag="f1")
            for dc in range(NDC):
                nc.tensor.matmul(
                    r(ynt_ps[:, dc, :]),
                    r(yn[:, dc * 128 : (dc + 1) * 128]),
                    r(ident_sb[:HP, :HP]),
                    is_transpose=True,
                    start=(dc == 0),
                    stop=(dc == NDC - 1),
                )
            ynt = fp.tile([128, NDC, HP], F32R, tag="ynt")
            nc.vector.tensor_copy(ynt, ynt_ps)

            # proto_tokens [P, D] = Yn @ WvT (per head) + bias_v
            proto_ps = fps.tile([P, D], F32, tag="f2")
            for h in range(H):
                for dc in range(NDC):
                    nc.tensor.matmul(
                        proto_ps[:, h * 128 : (h + 1) * 128],
                        r(ynt[:, dc, h * P : (h + 1) * P]),
                        r(wvT_sb[:, dc, h * 128 : (h + 1) * 128]),
                        start=(h == 0 and dc == 0),
                        stop=False,
                    )
            nc.tensor.matmul(proto_ps, r(ones24_sb), r(biasv_sb), start=False, stop=True)
            proto_sb = fp.tile([P, D], F32, tag="proto")
            nc.vector.tensor_copy(proto_sb, proto_ps)

            # proto_scores + rank-count top-k mask
            junk = fp.tile([P, D], F32, tag="junk")
            psc = fp.tile([P, 1], F32, tag="psc")
            nc.vector.scalar_tensor_tensor(
                junk, proto_sb, 1.0, proto_sb, op0=OP.mult, op1=OP.mult, accum_out=psc
            )
            pscT_ps = fps.tile([1, P], F32, tag="f3")
            nc.tensor.transpose(pscT_ps, psc, ident_sb[:P, :P].bitcast(F32))
            pscT = fp.tile([1, P], F32, tag="psct")
            nc.vector.tensor_copy(pscT, pscT_ps)
            g_ps = fps.tile([P, P], F32, tag="f4")
            nc.tensor.matmul(g_ps, ones24_sb.bitcast(F32), pscT, start=True, stop=True)
            junk2 = fp.tile([P, P], F32, tag="junk2")
            rank = fp.tile([P, 1], F32, tag="rank")
            nc.vector.scalar_tensor_tensor(
                junk2, g_ps, psc, onesPP_sb, op0=OP.is_gt, op1=OP.mult, accum_out=rank
            )
            mask = fp.tile([P, 1], F32, tag="mask")
            nc.vector.tensor_scalar(
                mask, rank, float(TOPK) - 0.5, None, op0=OP.is_lt
            )

            # z_T [128, NDC] = (proto^T @ mask) / TOPK
            z_ps = fps.tile([128, NDC], F32, tag="f5")
            for dc in range(NDC):
                nc.tensor.matmul(
                    z_ps[:, dc : dc + 1],
                    proto_sb[:, dc * 128 : (dc + 1) * 128],
                    mask,
                    start=(dc == 0),
                    stop=(dc == NDC - 1),
                )
            zT = fp.tile([128, NDC], F32, tag="zt")
            nc.scalar.activation(zT, z_ps, AF.Copy, scale=1.0 / TOPK)

            # h1_T [128, NDC] = W1 @ z  (columnwise)
            h1_ps = fps.tile([128, NDC], F32, tag="f6")
            for jc in range(NDC):
                for dc in range(NDC):
                    nc.tensor.matmul(
                        h1_ps[:, jc : jc + 1],
                        w1T_sb[:, dc, jc * 128 : (jc + 1) * 128],
                        zT[:, dc : dc + 1],
                        start=(jc == 0 and dc == 0),
                        stop=(jc == NDC - 1 and dc == NDC - 1),
                    )
            h1b = fp.tile([128, NDC], F32, tag="h1b")
            nc.vector.tensor_add(h1b, h1_ps, b1T_sb)
            h1g = fp.tile([128, NDC], F32, tag="h1g")
            nc.scalar.activation(h1g, h1b, AF.Sigmoid)
            h1s = fp.tile([128, NDC], F32, tag="h1s")
            nc.vector.tensor_mul(h1s, h1b, h1g)

            # out_row [1, D] = h1 @ W2^T + b2
            out_ps = fps.tile([1, D], F32, tag="f7")
            for dc in range(NDC):
                nc.tensor.matmul(
                    out_ps,
                    h1s[:, dc : dc + 1],
                    w2T_sb[:, dc, :],
                    start=(dc == 0),
                    stop=False,
                )
            nc.tensor.matmul(out_ps, ones1_sb.bitcast(F32), b2row_sb, start=False, stop=True)
            out_sb = fp.tile([1, D], F32, tag="out")
            nc.scalar.activation(out_sb, out_ps, AF.Copy)
            nc.sync.dma_start(y, out_sb)

    return nc


def make_host_weights(proto, ln_g, ln_b, Wq, Wk, Wv, W1, b1, W2, b2):
    f8 = np.float64
    g = ln_g.astype(f8)
    b = ln_b.astype(f8)
    Wk8, Wv8, Wq8 = Wk.astype(f8), Wv.astype(f8), Wq.astype(f8)
    Wk_eff = Wk8 * g[None, :]
    bias_k = Wk8 @ b
    Wv_eff = Wv8 * g[None, :]
    bias_v = Wv8 @ b
    Qp = proto.astype(f8) @ Wq8.T
    Qh = Qp.reshape(P, H, HD).transpose(1, 0, 2)
    nrm = np.maximum(np.linalg.norm(Qh, axis=-1, keepdims=True), 1e-12)
    Qh = Qh / nrm
    Mq = np.zeros((D, HP), f8)
    sbias = np.zeros((HP, 1), f8)
    for h in range(H):
        Mq[:, h * P : (h + 1) * P] = (Qh[h] @ Wk_eff[h * HD : (h + 1) * HD]).T / TEMP
        sbias[h * P : (h + 1) * P, 0] = Qh[h] @ bias_k[h * HD : (h + 1) * HD] / TEMP
    import ml_dtypes

    f = np.float32
    return {
        "wkT": np.ascontiguousarray(Wk_eff.T, f),
        "mq": np.ascontiguousarray(Mq, f),
        "sbias": sbias.astype(f),
        "biask": np.ascontiguousarray(bias_k.reshape(H, 128).T, f),
        "wvT": np.ascontiguousarray(Wv_eff.T, f),
        "biasv": np.ascontiguousarray(bias_v[None, :], f),
        "w1T": np.ascontiguousarray(W1.astype(f8).T, f),
        "b1T": np.ascontiguousarray(b1.astype(f8).reshape(NDC, 128).T, f),
        "w2T": np.ascontiguousarray(W2.astype(f8).T, f),
        "b2row": np.ascontiguousarray(b2.astype(f8)[None, :], f),
        "ident": np.eye(128, dtype=f),
        "selk": np.ascontiguousarray(
            np.broadcast_to(
                np.repeat(np.eye(H), P, axis=1)[None, :, :], (128, H, HP)
            ).astype(ml_dtypes.bfloat16)
        ),
        "ones24": np.ones((1, P), f),
        "ones1": np.ones((1, 1), f),
        "onesPP": np.ones((P, P), f),
    }


def _install_ntff_hook():
    """Recreate the missing antenv.axon_hooks registry and register the
    ctypes NTFF profile hook so run_bass_kernel_spmd(trace=True) works."""
    import types

    try:
        import antenv
    except ImportError:
        return False
    if "antenv.axon_hooks" not in sys.modules:
        m = types.ModuleType("antenv.axon_hooks")
        m._hook = None
        m.set_axon_ntff_profile_hook = lambda h: setattr(m, "_hook", h)
        m.get_axon_ntff_profile_hook = lambda: m._hook
        sys.modules["antenv.axon_hooks"] = m
        antenv.axon_hooks = m
    m = sys.modules["antenv.axon_hooks"]
    if m.get_axon_ntff_profile_hook() is None:
        try:
            from trn_agent_boot.trn_boot import _ntff_profile_via_ctypes

            hook = _ntff_profile_via_ctypes("/opt/axon/libaxon_pjrt.so")
            if hook is not None:
                m.set_axon_ntff_profile_hook(hook)
        except Exception as e:  # noqa: BLE001
            print("ntff hook install failed:", e)
            return False
    return m.get_axon_ntff_profile_hook() is not None


def kernel(
    x, proto, ln_g, ln_b, Wq, Wk, Wv, W1, b1, W2, b2, _trace=False, _t_total=T
):
    x = np.ascontiguousarray(np.asarray(x, np.float32))
    shared = make_host_weights(
        *(np.asarray(a, np.float32) for a in (proto, ln_g, ln_b, Wq, Wk, Wv, W1, b1, W2, b2))
    )
    nc = bacc.Bacc("TRN2", target_bir_lowering=False, debug=False, enable_asserts=False)
    build(nc, _t_total)
    nc.compile()
    in_maps = [dict(shared, xb=np.ascontiguousarray(x[bi])) for bi in range(B)]
    if _trace:
        _trace = _install_ntff_hook()
    res = run_bass_kernel_spmd(nc, in_maps, core_ids=list(range(B)), trace=_trace)
    out = np.stack([res.results[c]["y"][0] for c in range(B)])
    if _trace:
        kernel.last_results = res
    return out
